# revision 1
# baseline (speedup 1.0000x reference)
"""Trainium2 Bass kernel for nn_Net_34763465294339.

Four single-channel VALID convs (K=25/49/97/193, 16 output channels each) on
x[16,1,256,256], each squared + spatially averaged / scale -> stack -> fold
16 channels into 8 by adding halves. Output [16,8,4] f32.

Sharding: data-parallel over batch, 2 images per core, weights replicated.

Conv-as-matmul (output-stationary):
  PSUM tile per 8-output-row block: partitions m=(s,o)=8x16=128, free
  n=(j,b)=2*S (both images column-interleaved). Contraction k=(t,dj) over T kernel
  rows (T*K<=128; K=193 splits dj into 2 chunks). Accumulate over base-row
  sweep q=0..Q-1 (r0=i0+q*T) in PSUM.

  All per-q weight matrices are AP-offset slices of one padded matrix per
  chunk: M[(t,dj),(u,o)] = w[o, t+qmaxT-u, dj] (zero outside [0,K)), with
  lhsT_q = M[:, u0:u0+8, :] at u0=qmaxT-q*T.

  rhs tiles are shifted-row im2col tiles DMA'd straight from DRAM with
  overlapping-read APs, rotating through a small pool (re-fetched per group).

  Post: per block, DVE tensor_tensor_reduce (square, scaled, free-dim sum)
  into a stage column; a tiny fp32 fold-matmul (ones matrix) folds the
  (s,o)->o%8 partitions; per-(conv,image) column reduce; one DMA out.
"""
import os

import numpy as np
import ml_dtypes

import concourse.bass as bass
import concourse.bacc as bacc
import concourse.mybir as mybir
from concourse.tile import TileContext
from concourse.bass_utils import run_bass_kernel_spmd

BF16 = mybir.dt.bfloat16
F32 = mybir.dt.float32

IMG = 256
NCORES = 8
BLOCK_I = 8  # output rows per psum block
GROUP = 8    # psum blocks in flight (8 PSUM banks)

# (K, T, scale)
CONVS = [(25, 4, 1.0), (49, 2, 2.0), (97, 1, 4.0), (193, 1, 8.0)]
# rhs rotating-pool bufs per conv tag (>= max tiles in flight + prefetch)
RHS_BUFS = {25: 24, 49: 40, 97: 72, 193: 72}


def _conv_cfg(K, T):
    S = IMG - K + 1
    Q = (K + 7) // T
    U = (Q - 1) * T + 8
    chunks = [(0, K)] if T * K <= 128 else [(0, 128), (128, K)]
    return S, Q, U, chunks


def _build_M(w, K, T, scale):
    """w: [16,K,K] fp32, pre-scaled by sqrt(1/(S^2*scale)) so the squared
    conv outputs sum directly to the scaled mean. Returns fp32 [T*Kc, U*16]
    per dj-chunk."""
    S, Q, U, chunks = _conv_cfg(K, T)
    w = w * np.sqrt(1.0 / (float(S) * S * scale), dtype=np.float32)
    qmaxT = (Q - 1) * T
    out = []
    for (lo, hi) in chunks:
        Kc = hi - lo
        M = np.zeros((T * Kc, U, 16), dtype=np.float32)
        for t in range(T):
            for u in range(U):
                di = t + qmaxT - u
                if 0 <= di < K:
                    M[t * Kc:(t + 1) * Kc, u, :] = w[:, di, lo:hi].T
        out.append(np.ascontiguousarray(M.reshape(T * Kc, U * 16)))
    return out


def _build_fold():
    F = np.zeros((128, 8), dtype=np.float32)
    for p in range(128):
        F[p, (p % 16) % 8] = 1.0
    return F


def _col_layout(convs):
    col_base = {}
    c = 0
    for (K, T, scale) in convs:
        nb = (IMG - K + 1) // BLOCK_I
        for b in range(2):
            col_base[(K, b)] = c
            c += nb
    return col_base, c


def _build_nc(convs):
    nc = bacc.Bacc("TRN2", target_bir_lowering=False)
    x = nc.dram_tensor("x", [IMG, IMG, 2], BF16, kind="ExternalInput")
    m_handles = {}
    for (K, T, scale) in convs:
        S, Q, U, chunks = _conv_cfg(K, T)
        for idx, (lo, hi) in enumerate(chunks):
            Kc = hi - lo
            m_handles[(K, idx)] = nc.dram_tensor(
                f"m{K}_{idx}", [T * Kc, U * 16], BF16, kind="ExternalInput")
    fold = nc.dram_tensor("fold", [128, 8], F32, kind="ExternalInput")
    out = nc.dram_tensor("out", [2, 8, 4], F32, kind="ExternalOutput")

    col_base, TOT = _col_layout(convs)

    with TileContext(nc) as tc:
        with tc.tile_pool(name="consts", bufs=1) as cpool, \
             tc.tile_pool(name="rhsp", bufs=2) as rpool, \
             tc.tile_pool(name="scrp", bufs=4) as spool, \
             tc.tile_pool(name="accp", bufs=8, space="PSUM") as ppool:
            m_sb = {}
            for (K, idx), h in m_handles.items():
                mt = cpool.tile(list(h.shape), BF16, name=f"msb{K}_{idx}",
                                tag=f"m{K}_{idx}")
                nc.sync.dma_start(out=mt[:], in_=h[:])
                m_sb[(K, idx)] = mt
            fold_sb = cpool.tile([128, 8], F32, name="fold_sb", tag="fold")
            nc.sync.dma_start(out=fold_sb[:], in_=fold[:])
            stage = cpool.tile([128, TOT], F32, name="stage", tag="stage")

            for (K, T, scale) in convs:
                S, Q, U, chunks = _conv_cfg(K, T)
                qmaxT = (Q - 1) * T
                nb = S // BLOCK_I
                n = 2 * S
                nchunks = len(chunks)
                for g0 in range(0, nb, GROUP):
                    gblocks = list(range(g0, min(g0 + GROUP, nb)))
                    tiles = {}
                    psums = {}
                    for blk in gblocks:
                        psums[blk] = ppool.tile([128, n], F32,
                                                name=f"ps{K}_{blk}", tag="acc")
                    for q in range(Q):
                        u0 = qmaxT - q * T
                        for ci_, (lo, hi) in enumerate(chunks):
                            Kc = hi - lo
                            for blk in gblocks:
                                r0 = blk * BLOCK_I + q * T
                                key = (r0, ci_)
                                rt = tiles.get(key)
                                if rt is None:
                                    rt = rpool.tile(
                                        [T * Kc, n], BF16,
                                        name=f"r{K}_{r0}_{ci_}",
                                        tag=f"rhs{K}_{ci_}", bufs=RHS_BUFS[K])
                                    src = bass.AP(
                                        x, (r0 * IMG + lo) * 2,
                                        [[IMG * 2, T], [2, Kc], [1, n]])
                                    nc.gpsimd.dma_start(out=rt[:], in_=src)
                                    tiles[key] = rt
                                lhsT = m_sb[(K, ci_)].rearrange(
                                    "k (u o) -> k u o", o=16)[:, u0:u0 + 8, :]
                                nc.tensor.matmul(
                                    psums[blk][:], lhsT, rt[:],
                                    start=(q == 0 and ci_ == 0),
                                    stop=(q == Q - 1 and ci_ == nchunks - 1))
                    for blk in gblocks:
                        for b in range(2):
                            scr = spool.tile([128, S], F32,
                                             name=f"sq{K}_{blk}_{b}", tag="scr")
                            col = col_base[(K, b)] + blk
                            nc.scalar.activation(
                                out=scr[:],
                                in_=psums[blk][:, b::2],
                                func=mybir.ActivationFunctionType.Square,
                                accum_out=stage[:, col:col + 1])

            fold_ps = ppool.tile([8, TOT], F32, name="fold_ps", tag="acc")
            nc.tensor.matmul(fold_ps[:], fold_sb[:], stage[:],
                             start=True, stop=True)
            res = spool.tile([8, 8], F32, name="res", tag="res", bufs=1)
            for ci, (K, T, scale) in enumerate(CONVS):
                if (K, T, scale) not in convs:
                    continue
                nb = (IMG - K + 1) // BLOCK_I
                for b in range(2):
                    c0 = col_base[(K, b)]
                    oc = b * 4 + ci
                    nc.vector.reduce_sum(out=res[:8, oc:oc + 1],
                                         in_=fold_ps[:8, c0:c0 + nb],
                                         axis=mybir.AxisListType.X)
            dst = bass.AP(out, 0, [[4, 8], [32, 2], [1, 4]])
            nc.sync.dma_start(out=dst, in_=res[:8, :])
    return nc


_NC_CACHE = {}


def _get_nc(convs_key):
    if convs_key not in _NC_CACHE:
        nc = _build_nc(list(convs_key))
        nc.compile()
        _NC_CACHE[convs_key] = nc
    return _NC_CACHE[convs_key]


def kernel(x, w0, w1, w2, w3, _convs=None, _trace=False, _tmpdir=None):
    convs = CONVS if _convs is None else _convs
    ws = {25: w0, 49: w1, 97: w2, 193: w3}

    x = np.asarray(x, dtype=np.float32).reshape(16, IMG, IMG)
    xb = x.astype(ml_dtypes.bfloat16)

    shared = {}
    for (K, T, scale) in convs:
        w = np.asarray(ws[K], dtype=np.float32).reshape(16, K, K)
        for idx, M in enumerate(_build_M(w, K, T, scale)):
            shared[f"m{K}_{idx}"] = M.astype(ml_dtypes.bfloat16)
    shared["fold"] = _build_fold()

    in_maps = []
    for c in range(NCORES):
        m = dict(shared)
        m["x"] = np.ascontiguousarray(xb[2 * c:2 * c + 2].transpose(1, 2, 0))
        in_maps.append(m)

    nc = _get_nc(tuple(convs))
    kw = {}
    if _trace:
        kw.update(trace=True, tmpdir=_tmpdir)
    r = run_bass_kernel_spmd(nc, in_maps, list(range(NCORES)), **kw)
    out = np.concatenate([np.asarray(r.results[c]["out"], dtype=np.float32)
                          for c in range(NCORES)], axis=0)
    if _trace:
        kernel.last_exec_time_ns = r.exec_time_ns
        kernel.last_results = r
    return out



# revision 5
# speedup vs baseline: 34.7236x; 34.7236x over previous
"""Trainium2 Bass kernel for nn_Net_34763465294339.

Four single-channel VALID convs (K=25/49/97/193, 16 output channels each) on
x[16,1,256,256], each squared + spatially averaged / scale -> stack -> fold
16 channels into 8 by adding halves. Output [16,8,4] f32.

Sharding: data-parallel over batch, 2 images per core, weights replicated.

Conv-as-matmul (output-stationary):
  PSUM tile per 8-output-row block: partitions m=(s,o)=8x16=128, free
  n=(j,b)=2*S (both images column-interleaved). Contraction k=(t,dj) over T kernel
  rows (T*K<=128; K=193 splits dj into 2 chunks). Accumulate over base-row
  sweep q=0..Q-1 (r0=i0+q*T) in PSUM.

  All per-q weight matrices are AP-offset slices of one padded matrix per
  chunk: M[(t,dj),(u,o)] = w[o, t+qmaxT-u, dj] (zero outside [0,K)), with
  lhsT_q = M[:, u0:u0+8, :] at u0=qmaxT-q*T.

  rhs tiles are shifted-row im2col tiles DMA'd straight from DRAM with
  overlapping-read APs, rotating through a small pool (re-fetched per group).

  Post: per block, DVE tensor_tensor_reduce (square, scaled, free-dim sum)
  into a stage column; a tiny fp32 fold-matmul (ones matrix) folds the
  (s,o)->o%8 partitions; per-(conv,image) column reduce; one DMA out.
"""
import os

import numpy as np
import ml_dtypes

import concourse.bass as bass
import concourse.bacc as bacc
import concourse.mybir as mybir
from concourse.tile import TileContext
from concourse.bass_utils import run_bass_kernel_spmd

BF16 = mybir.dt.bfloat16
F32 = mybir.dt.float32

IMG = 256
NCORES = 8
BLOCK_I = 8  # output rows per psum block
GROUP = 8    # psum blocks in flight (8 PSUM banks)

# (K, T, scale)
CONVS = [(25, 4, 1.0), (49, 2, 2.0), (97, 1, 4.0), (193, 1, 8.0)]
# rhs rotating-pool bufs per conv tag (>= max tiles in flight + prefetch)
RHS_BUFS = {25: 24, 49: 40, 97: 72, 193: 72}


def _conv_cfg(K, T):
    S = IMG - K + 1
    Q = (K + 7) // T
    U = (Q - 1) * T + 8
    chunks = [(0, K)] if T * K <= 128 else [(0, 128), (128, K)]
    return S, Q, U, chunks


def _build_M(w, K, T, scale):
    """w: [16,K,K] fp32, pre-scaled by sqrt(1/(S^2*scale)) so the squared
    conv outputs sum directly to the scaled mean. Returns fp32 [T*Kc, U*16]
    per dj-chunk."""
    S, Q, U, chunks = _conv_cfg(K, T)
    w = w * np.sqrt(1.0 / (float(S) * S * scale), dtype=np.float32)
    qmaxT = (Q - 1) * T
    out = []
    for (lo, hi) in chunks:
        Kc = hi - lo
        M = np.zeros((T * Kc, U, 16), dtype=np.float32)
        for t in range(T):
            for u in range(U):
                di = t + qmaxT - u
                if 0 <= di < K:
                    M[t * Kc:(t + 1) * Kc, u, :] = w[:, di, lo:hi].T
        out.append(np.ascontiguousarray(M.reshape(T * Kc, U * 16)))
    return out


def _build_fold():
    F = np.zeros((128, 8), dtype=np.float32)
    for p in range(128):
        F[p, (p % 16) % 8] = 1.0
    return F


def _col_layout(convs):
    col_base = {}
    c = 0
    for (K, T, scale) in convs:
        nb = (IMG - K + 1) // BLOCK_I
        for b in range(2):
            col_base[(K, b)] = c
            c += nb
    return col_base, c


def _build_nc(convs, niter=1):
    nc = bacc.Bacc("TRN2", target_bir_lowering=False)
    x = nc.dram_tensor("x", [IMG, IMG, 2], BF16, kind="ExternalInput")
    m_handles = {}
    for (K, T, scale) in convs:
        S, Q, U, chunks = _conv_cfg(K, T)
        for idx, (lo, hi) in enumerate(chunks):
            Kc = hi - lo
            m_handles[(K, idx)] = nc.dram_tensor(
                f"m{K}_{idx}", [T * Kc, U * 16], BF16, kind="ExternalInput")
    fold = nc.dram_tensor("fold", [128, 8], F32, kind="ExternalInput")
    out = nc.dram_tensor("out", [2, 8, 4], F32, kind="ExternalOutput")

    col_base, TOT = _col_layout(convs)

    with TileContext(nc) as tc:
        for _it in range(niter):
            _build_iter(nc, tc, convs, x, m_handles, fold, out,
                        col_base, TOT, _it)
    return nc


def _build_iter(nc, tc, convs, x, m_handles, fold, out, col_base, TOT, it):
    if True:
        with tc.tile_pool(name=f"consts{it}", bufs=1) as cpool, \
             tc.tile_pool(name=f"rhsp{it}", bufs=2) as rpool, \
             tc.tile_pool(name=f"scrp{it}", bufs=4) as spool, \
             tc.tile_pool(name=f"accp{it}", bufs=8, space="PSUM") as ppool:
            m_sb = {}
            for (K, idx), h in m_handles.items():
                mt = cpool.tile(list(h.shape), BF16, name=f"msb{K}_{idx}",
                                tag=f"m{K}_{idx}")
                nc.sync.dma_start(out=mt[:], in_=h[:])
                m_sb[(K, idx)] = mt
            fold_sb = cpool.tile([128, 8], F32, name="fold_sb", tag="fold")
            nc.sync.dma_start(out=fold_sb[:], in_=fold[:])
            stage = cpool.tile([128, TOT], F32, name="stage", tag="stage")

            for (K, T, scale) in convs:
                S, Q, U, chunks = _conv_cfg(K, T)
                qmaxT = (Q - 1) * T
                nb = S // BLOCK_I
                n = 2 * S
                nchunks = len(chunks)
                for g0 in range(0, nb, GROUP):
                    gblocks = list(range(g0, min(g0 + GROUP, nb)))
                    tiles = {}
                    psums = {}
                    for blk in gblocks:
                        psums[blk] = ppool.tile([128, n], F32,
                                                name=f"ps{K}_{blk}", tag="acc")
                    for q in range(Q):
                        u0 = qmaxT - q * T
                        for ci_, (lo, hi) in enumerate(chunks):
                            Kc = hi - lo
                            for blk in gblocks:
                                r0 = blk * BLOCK_I + q * T
                                key = (r0, ci_)
                                rt = tiles.get(key)
                                if rt is None:
                                    rt = rpool.tile(
                                        [T * Kc, n], BF16,
                                        name=f"r{K}_{r0}_{ci_}",
                                        tag=f"rhs{K}_{ci_}", bufs=RHS_BUFS[K])
                                    src = bass.AP(
                                        x, (r0 * IMG + lo) * 2,
                                        [[IMG * 2, T], [2, Kc], [1, n]])
                                    nc.gpsimd.dma_start(out=rt[:], in_=src)
                                    tiles[key] = rt
                                lhsT = m_sb[(K, ci_)].rearrange(
                                    "k (u o) -> k u o", o=16)[:, u0:u0 + 8, :]
                                nc.tensor.matmul(
                                    psums[blk][:], lhsT, rt[:],
                                    start=(q == 0 and ci_ == 0),
                                    stop=(q == Q - 1 and ci_ == nchunks - 1))
                    for blk in gblocks:
                        for b in range(2):
                            scr = spool.tile([128, S], F32,
                                             name=f"sq{K}_{blk}_{b}", tag="scr")
                            col = col_base[(K, b)] + blk
                            nc.scalar.activation(
                                out=scr[:],
                                in_=psums[blk][:, b::2],
                                func=mybir.ActivationFunctionType.Square,
                                accum_out=stage[:, col:col + 1])

            fold_ps = ppool.tile([8, TOT], F32, name="fold_ps", tag="acc")
            nc.tensor.matmul(fold_ps[:], fold_sb[:], stage[:],
                             start=True, stop=True)
            res = spool.tile([8, 8], F32, name="res", tag="res", bufs=1)
            for ci, (K, T, scale) in enumerate(CONVS):
                if (K, T, scale) not in convs:
                    continue
                nb = (IMG - K + 1) // BLOCK_I
                for b in range(2):
                    c0 = col_base[(K, b)]
                    oc = b * 4 + ci
                    nc.vector.reduce_sum(out=res[:8, oc:oc + 1],
                                         in_=fold_ps[:8, c0:c0 + nb],
                                         axis=mybir.AxisListType.X)
            dst = bass.AP(out, 0, [[4, 8], [32, 2], [1, 4]])
            nc.sync.dma_start(out=dst, in_=res[:8, :])


_NC_CACHE = {}


def _get_nc(convs_key, niter=1):
    key = (convs_key, niter)
    if key not in _NC_CACHE:
        nc = _build_nc(list(convs_key), niter=niter)
        nc.compile()
        _NC_CACHE[key] = nc
    return _NC_CACHE[key]


def make_in_maps(inputs, convs=None):
    convs = CONVS if convs is None else convs
    ws = {25: inputs["w0"], 49: inputs["w1"],
          97: inputs["w2"], 193: inputs["w3"]}

    x = np.asarray(inputs["x"], dtype=np.float32).reshape(16, IMG, IMG)
    xb = x.astype(ml_dtypes.bfloat16)

    shared = {}
    for (K, T, scale) in convs:
        w = np.asarray(ws[K], dtype=np.float32).reshape(16, K, K)
        for idx, M in enumerate(_build_M(w, K, T, scale)):
            shared[f"m{K}_{idx}"] = M.astype(ml_dtypes.bfloat16)
    shared["fold"] = _build_fold()

    in_maps = []
    for c in range(NCORES):
        m = dict(shared)
        m["x"] = np.ascontiguousarray(xb[2 * c:2 * c + 2].transpose(1, 2, 0))
        in_maps.append(m)
    return in_maps


def kernel(x, w0, w1, w2, w3, _convs=None, _trace=False, _tmpdir=None):
    convs = CONVS if _convs is None else _convs
    in_maps = make_in_maps(
        dict(x=x, w0=w0, w1=w1, w2=w2, w3=w3), convs)

    nc = _get_nc(tuple(convs))
    kw = {}
    if _trace:
        kw.update(trace=True, tmpdir=_tmpdir)
    r = run_bass_kernel_spmd(nc, in_maps, list(range(NCORES)), **kw)
    out = np.concatenate([np.asarray(r.results[c]["out"], dtype=np.float32)
                          for c in range(NCORES)], axis=0)
    if _trace:
        kernel.last_exec_time_ns = r.exec_time_ns
        kernel.last_results = r
    return out



# revision 7
# speedup vs baseline: 76.5782x; 2.2054x over previous
"""Trainium2 Bass kernel for nn_Net_34763465294339.

Four single-channel VALID convs (K=25/49/97/193, 16 output channels each) on
x[16,1,256,256], each squared + spatially averaged / scale -> stack -> fold
16 channels into 8 by adding halves. Output [16,8,4] f32.

Sharding: data-parallel over batch, 2 images per core, weights replicated.

fp8 DoubleRow row-slab formulation (per conv):
  One matmul per dj0 step per (block-group, image), accumulating in PSUM:
    out[(u,o), (blk,j)] += sum_{(r2,djs),i} W[(r2,djs),i,(u,o)] * X[(r2,djs),i,(blk,j)]
  Contraction partitions (r2, djs): r2 indexes image row PAIRS (DoubleRow
  pair dim i = row parity), djs a dj subgroup. The rhs tile Xg holds raw
  256-byte planar row slices x[b, 2*r2+i+8*(grp*NB+blk), djs:djs+256] --
  full rows, so ONE tile per (group,image) serves every dj0 step via an AP
  column offset of dj0*DJS. Weights W[(r2,djs), i, (u,o)] =
  wq[o, 2*r2+i-u, dj0*DJS+djs] (zero outside ranges) are per-dj0 matrices
  built on host in fp8 (pre-scaled by a per-conv power of two).

  Per-conv normalization (1/(S^2*scale*SW^2)) folds into the ScalarE
  Square activation's input scale. Post: per-(block,image) square+reduce
  into a stage column, ones-matmul fold 16ch->8, DVE column reduce, DMA out.
"""
import numpy as np
import ml_dtypes

import concourse.bass as bass
import concourse.bacc as bacc
import concourse.mybir as mybir
from concourse.tile import TileContext
from concourse.bass_utils import run_bass_kernel_spmd

F32 = mybir.dt.float32
FP8 = mybir.dt.float8e4
NPFP8 = ml_dtypes.float8_e4m3

IMG = 256
NCORES = 8
ROWB = IMG + 1  # planar x padded with one zero row per image

# (K, scale_ref); order fixed = output feature order
CONVS = [(25, 1.0), (49, 2.0), (97, 4.0), (193, 8.0)]
# per-conv pow2 weight scale into fp8 sweet spot (w sigmas .05/.02/.01/.005)
SW = {25: 16.0, 49: 64.0, 97: 128.0, 193: 256.0}


def _cfg(K):
    S = IMG - K + 1
    nb = S // 8
    r2 = (K + 7) // 2           # row pairs in the di band
    DJS = max(1, 128 // r2)     # dj subgroups packed into contraction
    steps = -(-K // DJS)        # dj0 steps
    NB = min(512 // S, nb)      # blocks per matmul (psum free cap 512 f32)
    ngrp = -(-nb // NB)
    return S, nb, r2, DJS, steps, NB, ngrp


def _build_w8(wq, K):
    """wq: [16,K,K] f32 already scaled. Returns [r2*DJS, steps*2*8*16] fp8
    with value at ((r2,djs), dj0, i, u, o) = wq[o, 2*r2+i-u, dj0*DJS+djs]."""
    S, nb, r2, DJS, steps, NB, ngrp = _cfg(K)
    M = np.zeros((r2, DJS, steps, 2, 8, 16), dtype=np.float32)
    for p in range(r2):
        for i in range(2):
            for u in range(8):
                di = 2 * p + i - u
                if not (0 <= di < K):
                    continue
                # M[p, djs, dj0, i, u, :] = wq[:, di, dj0*DJS+djs].T
                w_slice = wq[:, di, :]  # [16, K]
                dj = np.arange(steps * DJS)
                valid = dj < K
                dst = np.zeros((steps * DJS, 16), dtype=np.float32)
                dst[valid] = w_slice[:, dj[valid]].T
                M[p, :, :, i, u, :] = dst.reshape(steps, DJS, 16).transpose(1, 0, 2)
    return np.ascontiguousarray(
        M.reshape(r2 * DJS, steps * 2 * 8 * 16)).astype(NPFP8)


def _build_fold():
    F = np.zeros((128, 8), dtype=np.float32)
    for p in range(128):
        F[p, (p % 16) % 8] = 1.0
    return F


def _col_layout(convs):
    col_base = {}
    c = 0
    for (K, scale) in convs:
        nb = (IMG - K + 1) // 8
        for b in range(2):
            col_base[(K, b)] = c
            c += nb
    return col_base, c


def _build_nc(convs, niter=1):
    nc = bacc.Bacc("TRN2", target_bir_lowering=False)
    x = nc.dram_tensor("x", [2, ROWB, IMG], FP8, kind="ExternalInput")
    m_handles = {}
    for (K, scale) in convs:
        S, nb, r2, DJS, steps, NB, ngrp = _cfg(K)
        m_handles[K] = nc.dram_tensor(
            f"m{K}", [r2 * DJS, steps * 256], FP8, kind="ExternalInput")
    fold = nc.dram_tensor("fold", [128, 8], F32, kind="ExternalInput")
    out = nc.dram_tensor("out", [2, 8, 4], F32, kind="ExternalOutput")

    col_base, TOT = _col_layout(convs)

    with TileContext(nc) as tc:
        for _it in range(niter):
            _build_iter(nc, tc, convs, x, m_handles, fold, out,
                        col_base, TOT, _it)
    return nc


def _build_iter(nc, tc, convs, x, m_handles, fold, out, col_base, TOT, it):
    with tc.tile_pool(name=f"consts{it}", bufs=1) as cpool, \
         tc.tile_pool(name=f"xgp{it}", bufs=2) as xpool, \
         tc.tile_pool(name=f"scrp{it}", bufs=4) as spool, \
         tc.tile_pool(name=f"accp{it}", bufs=8, space="PSUM") as ppool:
        m_sb = {}
        for K, h in m_handles.items():
            mt = cpool.tile(list(h.shape), FP8, name=f"msb{K}", tag=f"m{K}")
            nc.sync.dma_start(out=mt[:], in_=h[:])
            m_sb[K] = mt
        fold_sb = cpool.tile([128, 8], F32, name="fold_sb", tag="fold")
        nc.sync.dma_start(out=fold_sb[:], in_=fold[:])
        stage = cpool.tile([128, TOT], F32, name="stage", tag="stage")

        for (K, scale) in convs:
            S, nb, r2, DJS, steps, NB, ngrp = _cfg(K)
            s_act = 1.0 / (SW[K] * S * float(np.sqrt(scale)))
            for grp in range(ngrp):
                nbact = min(NB, nb - grp * NB)
                for b in range(2):
                    # Xg: partitions (r2, djs); free [i][blk][256B row slice]
                    xg = xpool.tile([r2 * DJS, 2 * nbact * 256], FP8,
                                    name=f"xg{K}_{grp}_{b}",
                                    tag=f"xg{K}_{b}_{nbact}", bufs=3)
                    xga = xg[:]
                    for i in range(2):
                        for blk in range(nbact):
                            src = bass.AP(
                                x, b * (ROWB * IMG)
                                + ((grp * NB + blk) * 8 + i) * IMG,
                                [[2 * IMG, r2], [1, DJS], [1, 256]])
                            dst = bass.AP(
                                xga.tensor,
                                xga.offset + (i * nbact + blk) * 256,
                                [xga.ap[0], [1, 256]])
                            nc.sync.dma_start(out=dst, in_=src)
                    ps = ppool.tile([128, nbact * S], F32,
                                    name=f"ps{K}_{grp}_{b}", tag="acc")
                    mta = m_sb[K][:]
                    for dj0 in range(steps):
                        lhsT = bass.AP(
                            mta.tensor, mta.offset + dj0 * 256,
                            [mta.ap[0], [128, 2], [1, 128]])
                        rhs = bass.AP(
                            xga.tensor, xga.offset + dj0 * DJS,
                            [xga.ap[0], [nbact * 256, 2],
                             [256, nbact], [1, S]])
                        nc.tensor.matmul(
                            ps[:], lhsT, rhs,
                            start=(dj0 == 0), stop=(dj0 == steps - 1),
                            perf_mode=mybir.MatmulPerfMode.DoubleRow)
                    for blk in range(nbact):
                        scr = spool.tile([128, S], F32,
                                         name=f"sq{K}_{grp}_{b}_{blk}",
                                         tag="scr")
                        col = col_base[(K, b)] + grp * NB + blk
                        nc.scalar.activation(
                            out=scr[:],
                            in_=ps[:, blk * S:(blk + 1) * S],
                            func=mybir.ActivationFunctionType.Square,
                            scale=float(s_act),
                            accum_out=stage[:, col:col + 1])

        fold_ps = ppool.tile([8, TOT], F32, name="fold_ps", tag="acc")
        nc.tensor.matmul(fold_ps[:], fold_sb[:], stage[:],
                         start=True, stop=True)
        res = spool.tile([8, 8], F32, name="res", tag="res", bufs=1)
        for ci, (K, scale) in enumerate(CONVS):
            if (K, scale) not in convs:
                continue
            nb = (IMG - K + 1) // 8
            for b in range(2):
                c0 = col_base[(K, b)]
                oc = b * 4 + ci
                nc.vector.reduce_sum(out=res[:8, oc:oc + 1],
                                     in_=fold_ps[:8, c0:c0 + nb],
                                     axis=mybir.AxisListType.X)
        dst = bass.AP(out, 0, [[4, 8], [32, 2], [1, 4]])
        nc.sync.dma_start(out=dst, in_=res[:8, :])


_NC_CACHE = {}


def _get_nc(convs_key, niter=1):
    key = (convs_key, niter)
    if key not in _NC_CACHE:
        nc = _build_nc(list(convs_key), niter=niter)
        nc.compile()
        _NC_CACHE[key] = nc
    return _NC_CACHE[key]


def make_in_maps(inputs, convs=None):
    convs = CONVS if convs is None else convs
    ws = {25: inputs["w0"], 49: inputs["w1"],
          97: inputs["w2"], 193: inputs["w3"]}

    x = np.asarray(inputs["x"], dtype=np.float32).reshape(16, IMG, IMG)

    shared = {}
    for (K, scale) in convs:
        w = np.asarray(ws[K], dtype=np.float32).reshape(16, K, K)
        wq = np.clip(w * SW[K], -240.0, 240.0)
        # quantize weights to fp8 once (matmul sees these exact values)
        wq = wq.astype(NPFP8).astype(np.float32)
        shared[f"m{K}"] = _build_w8(wq, K)
    shared["fold"] = _build_fold()

    in_maps = []
    for c in range(NCORES):
        m = dict(shared)
        xp = np.zeros((2, ROWB, IMG), dtype=NPFP8)
        xp[:, :IMG, :] = np.clip(x[2 * c:2 * c + 2], -240.0, 240.0
                                 ).astype(NPFP8)
        m["x"] = xp
        in_maps.append(m)
    return in_maps


def kernel(x, w0, w1, w2, w3, _convs=None):
    convs = CONVS if _convs is None else _convs
    in_maps = make_in_maps(dict(x=x, w0=w0, w1=w1, w2=w2, w3=w3), convs)
    nc = _get_nc(tuple(convs))
    r = run_bass_kernel_spmd(nc, in_maps, list(range(NCORES)))
    out = np.concatenate([np.asarray(r.results[c]["out"], dtype=np.float32)
                          for c in range(NCORES)], axis=0)
    return out


# revision 15
# speedup vs baseline: 120.1835x; 1.5694x over previous
"""Trainium2 Bass kernel for nn_Net_34763465294339.

Four single-channel VALID convs (K=25/49/97/193, 16 output channels each) on
x[16,1,256,256], each squared + spatially averaged / scale -> stack -> fold
16 channels into 8 by adding halves. Output [16,8,4] f32.

Sharding: data-parallel over batch, 2 images per core, weights replicated.

fp8 DoubleRow row-slab formulation (per conv):
  One matmul per dj0 step per (block-group, image), accumulating in PSUM:
    out[(u,o), (blk,j)] += sum_{(r2,djs),i} W[(r2,djs),i,(u,o)] * X[(r2,djs),i,(blk,j)]
  Contraction partitions (r2, djs): r2 indexes image row PAIRS (DoubleRow
  pair dim i = row parity), djs a dj subgroup. The rhs tile Xg holds raw
  256-byte planar row slices x[b, 2*r2+i+8*(grp*NB+blk), djs:djs+256] --
  full rows, so ONE tile per (group,image) serves every dj0 step via an AP
  column offset of dj0*DJS. Weights W[(r2,djs), i, (u,o)] =
  wq[o, 2*r2+i-u, dj0*DJS+djs] (zero outside ranges) are per-dj0 matrices
  built on host in fp8 (pre-scaled by a per-conv power of two).

  Per-conv normalization (1/(S^2*scale*SW^2)) folds into the ScalarE
  Square activation's input scale. Post: per-(block,image) square+reduce
  into a stage column, ones-matmul fold 16ch->8, DVE column reduce, DMA out.
"""
import numpy as np
import ml_dtypes

import concourse.bass as bass
import concourse.bacc as bacc
import concourse.mybir as mybir
from concourse.tile import TileContext
from concourse.bass_utils import run_bass_kernel_spmd

F32 = mybir.dt.float32
FP8 = mybir.dt.float8e4
NPFP8 = ml_dtypes.float8_e4m3

IMG = 256
NCORES = 8
ROWB = IMG + 1  # planar x padded with one zero row per image

# (K, scale_ref); processing order (heavy convs first). Output feature
# order is fixed by CI below, independent of processing order.
CONVS = [(193, 8.0), (97, 4.0), (49, 2.0), (25, 1.0)]
CI = {25: 0, 49: 1, 97: 2, 193: 3}
# per-conv pow2 weight scale into fp8 sweet spot (w sigmas .05/.02/.01/.005)
SW = {25: 16.0, 49: 64.0, 97: 128.0, 193: 256.0}


def _cfg(K):
    S = IMG - K + 1
    nb = S // 8
    r2 = (K + 7) // 2           # row pairs in the di band
    DJS = max(1, 128 // r2)     # dj subgroups packed into contraction
    steps = -(-K // DJS)        # dj0 steps
    NB = min(512 // S, nb)      # blocks per matmul (psum free cap 512 f32)
    ngrp = -(-nb // NB)
    return S, nb, r2, DJS, steps, NB, ngrp


def _build_w8(wq, K):
    """wq: [16,K,K] f32 already scaled. Returns [r2*DJS, steps*2*8*16] fp8
    with value at ((r2,djs), dj0, i, u, o) = wq[o, 2*r2+i-u, dj0*DJS+djs]."""
    S, nb, r2, DJS, steps, NB, ngrp = _cfg(K)
    M = np.zeros((r2, DJS, steps, 2, 8, 16), dtype=np.float32)
    for p in range(r2):
        for i in range(2):
            for u in range(8):
                di = 2 * p + i - u
                if not (0 <= di < K):
                    continue
                # M[p, djs, dj0, i, u, :] = wq[:, di, dj0*DJS+djs].T
                w_slice = wq[:, di, :]  # [16, K]
                dj = np.arange(steps * DJS)
                valid = dj < K
                dst = np.zeros((steps * DJS, 16), dtype=np.float32)
                dst[valid] = w_slice[:, dj[valid]].T
                M[p, :, :, i, u, :] = dst.reshape(steps, DJS, 16).transpose(1, 0, 2)
    return np.ascontiguousarray(
        M.reshape(r2 * DJS, steps * 2 * 8 * 16)).astype(NPFP8)


def _build_fold():
    F = np.zeros((128, 8), dtype=np.float32)
    for p in range(128):
        F[p, (p % 16) % 8] = 1.0
    return F


def _col_layout(convs):
    col_base = {}
    c = 0
    for (K, scale) in convs:
        nb = (IMG - K + 1) // 8
        for b in range(2):
            col_base[(K, b)] = c
            c += nb
    return col_base, c


def _build_nc(convs, niter=1):
    nc = bacc.Bacc("TRN2", target_bir_lowering=False)
    x = nc.dram_tensor("x", [2, ROWB, IMG], FP8, kind="ExternalInput")
    m_handles = {}
    for (K, scale) in convs:
        S, nb, r2, DJS, steps, NB, ngrp = _cfg(K)
        m_handles[K] = nc.dram_tensor(
            f"m{K}", [r2 * DJS, steps * 256], FP8, kind="ExternalInput")
    fold = nc.dram_tensor("fold", [128, 8], F32, kind="ExternalInput")
    out = nc.dram_tensor("out", [2, 8, 4], F32, kind="ExternalOutput")

    col_base, TOT = _col_layout(convs)

    with TileContext(nc) as tc:
        for _it in range(niter):
            _build_iter(nc, tc, convs, x, m_handles, fold, out,
                        col_base, TOT, _it)
    return nc


def _build_iter(nc, tc, convs, x, m_handles, fold, out, col_base, TOT, it):
    with tc.tile_pool(name=f"consts{it}", bufs=1) as cpool, \
         tc.tile_pool(name=f"xgp{it}", bufs=2) as xpool, \
         tc.tile_pool(name=f"scrp{it}", bufs=4) as spool, \
         tc.tile_pool(name=f"accp{it}", bufs=8, space="PSUM") as ppool:
        m_sb = {}
        for K, h in m_handles.items():
            mt = cpool.tile(list(h.shape), FP8, name=f"msb{K}", tag=f"m{K}")
            nc.sync.dma_start(out=mt[:], in_=h[:])
            m_sb[K] = mt
        fold_sb = cpool.tile([128, 8], F32, name="fold_sb", tag="fold")
        nc.sync.dma_start(out=fold_sb[:], in_=fold[:])
        stage = cpool.tile([128, TOT], F32, name="stage", tag="stage")

        for (K, scale) in convs:
            S, nb, r2, DJS, steps, NB, ngrp = _cfg(K)
            s_act = 1.0 / (SW[K] * S * float(np.sqrt(scale)))
            mta = m_sb[K][:]
            pairs = [(grp, b) for grp in range(ngrp) for b in range(2)]
            # dj0-outer over sets of 4 psum groups: consecutive matmuls
            # share the stationary operand, amortizing LDWEIGHTS
            for s0 in range(0, len(pairs), 4):
                gset = pairs[s0:s0 + 4]
                xgs, pss, nbacts = {}, {}, {}
                for (grp, b) in gset:
                    nbact = min(NB, nb - grp * NB)
                    nbacts[(grp, b)] = nbact
                    ntag = sum(1 for g in range(ngrp)
                               if min(NB, nb - g * NB) == nbact)
                    # Xg: partitions (r2,djs); free [i][blk][256B row slice]
                    xg = xpool.tile([r2 * DJS, 2 * nbact * 256], FP8,
                                    name=f"xg{K}_{grp}_{b}",
                                    tag=f"xg{K}_{b}_{nbact}",
                                    bufs=min(8, 2 * ntag + 1))
                    xga = xg[:]
                    for i in range(2):
                        for blk in range(nbact):
                            src = bass.AP(
                                x, b * (ROWB * IMG)
                                + ((grp * NB + blk) * 8 + i) * IMG,
                                [[2 * IMG, r2], [1, DJS], [1, 256]])
                            dst = bass.AP(
                                xga.tensor,
                                xga.offset + (i * nbact + blk) * 256,
                                [xga.ap[0], [1, 256]])
                            nc.sync.dma_start(out=dst, in_=src)
                    xgs[(grp, b)] = xga
                    pss[(grp, b)] = ppool.tile(
                        [128, nbact * S], F32,
                        name=f"ps{K}_{grp}_{b}", tag="acc")
                for dj0 in range(steps):
                    lhsT = bass.AP(
                        mta.tensor, mta.offset + dj0 * 256,
                        [mta.ap[0], [128, 2], [1, 128]])
                    for (grp, b) in gset:
                        xga = xgs[(grp, b)]
                        nbact = nbacts[(grp, b)]
                        rhs = bass.AP(
                            xga.tensor, xga.offset + dj0 * DJS,
                            [xga.ap[0], [nbact * 256, 2],
                             [256, nbact], [1, S]])
                        nc.tensor.matmul(
                            pss[(grp, b)][:], lhsT, rhs,
                            start=(dj0 == 0), stop=(dj0 == steps - 1),
                            perf_mode=mybir.MatmulPerfMode.DoubleRow)
                for (grp, b) in gset:
                    nbact = nbacts[(grp, b)]
                    ps = pss[(grp, b)]
                    for blk in range(nbact):
                        scr = spool.tile([128, S], F32,
                                         name=f"sq{K}_{grp}_{b}_{blk}",
                                         tag="scr")
                        col = col_base[(K, b)] + grp * NB + blk
                        nc.scalar.activation(
                            out=scr[:],
                            in_=ps[:, blk * S:(blk + 1) * S],
                            func=mybir.ActivationFunctionType.Square,
                            scale=float(s_act),
                            accum_out=stage[:, col:col + 1])

        fold_ps = ppool.tile([8, TOT], F32, name="fold_ps", tag="acc")
        nc.tensor.matmul(fold_ps[:], fold_sb[:], stage[:],
                         start=True, stop=True)
        res = spool.tile([8, 8], F32, name="res", tag="res", bufs=1)
        for (K, scale) in convs:
            ci = CI[K]
            nb = (IMG - K + 1) // 8
            for b in range(2):
                c0 = col_base[(K, b)]
                oc = b * 4 + ci
                nc.vector.reduce_sum(out=res[:8, oc:oc + 1],
                                     in_=fold_ps[:8, c0:c0 + nb],
                                     axis=mybir.AxisListType.X)
        dst = bass.AP(out, 0, [[4, 8], [32, 2], [1, 4]])
        nc.sync.dma_start(out=dst, in_=res[:8, :])


_NC_CACHE = {}


def _get_nc(convs_key, niter=1):
    key = (convs_key, niter)
    if key not in _NC_CACHE:
        nc = _build_nc(list(convs_key), niter=niter)
        nc.compile()
        _NC_CACHE[key] = nc
    return _NC_CACHE[key]


def make_in_maps(inputs, convs=None):
    convs = CONVS if convs is None else convs
    ws = {25: inputs["w0"], 49: inputs["w1"],
          97: inputs["w2"], 193: inputs["w3"]}

    x = np.asarray(inputs["x"], dtype=np.float32).reshape(16, IMG, IMG)

    shared = {}
    for (K, scale) in convs:
        w = np.asarray(ws[K], dtype=np.float32).reshape(16, K, K)
        wq = np.clip(w * SW[K], -240.0, 240.0)
        # quantize weights to fp8 once (matmul sees these exact values)
        wq = wq.astype(NPFP8).astype(np.float32)
        shared[f"m{K}"] = _build_w8(wq, K)
    shared["fold"] = _build_fold()

    in_maps = []
    for c in range(NCORES):
        m = dict(shared)
        xp = np.zeros((2, ROWB, IMG), dtype=NPFP8)
        xp[:, :IMG, :] = np.clip(x[2 * c:2 * c + 2], -240.0, 240.0
                                 ).astype(NPFP8)
        m["x"] = xp
        in_maps.append(m)
    return in_maps


def kernel(x, w0, w1, w2, w3, _convs=None):
    convs = CONVS if _convs is None else _convs
    in_maps = make_in_maps(dict(x=x, w0=w0, w1=w1, w2=w2, w3=w3), convs)
    nc = _get_nc(tuple(convs))
    r = run_bass_kernel_spmd(nc, in_maps, list(range(NCORES)))
    out = np.concatenate([np.asarray(r.results[c]["out"], dtype=np.float32)
                          for c in range(NCORES)], axis=0)
    return out
